# revision 12
# baseline (speedup 1.0000x reference)
"""BertSelfAttention TRN2 kernel (v2 — ACT-engine-saturated design).

Problem: B=4, S=2048, H=768, NH=12, HD=64, fp32.
Sharding: 8 cores; core c owns batch b = c//2 and head-group g = c%2
(6 heads = 384 hidden cols). Each core runs the same Bass program on its
shard; host reassembles.

Bottleneck analysis: the softmax exp is 6*S^2 = 25.2M elements per core
and can only run on the Activation engine (1 elem/cycle/partition
@1.2GHz) => ~199us floor. Everything else is organized to hide under
that: all matmuls in bf16 (1 col/cycle on PE), context accumulated in
natural [q, d] layout with a fused denominator column (no output
transposes), and the emission order software-pipelines scores(k+1)
between the two ctx halves of k so ACT never waits on PSUM buffers.

Per-core algorithm (fp32 PSUM, bf16 operands):
  xt = x^T, wt = W^T          (PE transposes of bf16 casts)
  QT/KT[pair] = W @ xT + b    (partitions = [headA d | headB d])
  V = x @ WvT + bv, scaled by w_k = exp(mask_k); augmented col w
  per (qtr, pair, ktp):
     scoresT[k, q] = K @ QT   (2 heads x 2 k-tiles, N=512)
     E = exp(SCALE * scoresT) (ACT, [128,1024] from PSUM -> bf16 SBUF)
     ctx[q, 65] += E^T @ v_aug  (natural layout, denom in col 64)
  epilogue: rec = 1/ctx[:,64]; out = ctx[:,0:64] * rec  (DVE only)
"""

import numpy as np

B, S, H = 4, 2048, 768
NH, HD = 12, 64
SCALE = 1.0 / np.sqrt(np.float32(HD)).astype(np.float32)
HPC = H // 2          # 384 hidden cols per core (6 heads)
NHEADS = 6            # heads per core
NPAIR = 3             # head pairs per core
NST = S // 128        # 16 S-tiles
NHT = H // 128        # 6 hidden tiles
NCORES = 8

_CACHE = {}


def build_nc(reps=1, loop_part="all"):
    import concourse.bacc as bacc
    import concourse.mybir as mybir
    import concourse.tile as tile
    from concourse.masks import make_identity

    f32 = mybir.dt.float32
    bf16 = mybir.dt.bfloat16
    AF = mybir.ActivationFunctionType
    OP = mybir.AluOpType

    nc = bacc.Bacc("TRN2", target_bir_lowering=False, debug=False,
                   num_devices=NCORES)

    x_d = nc.declare_dram_parameter("x", [S, H], f32, isOutput=False)
    wq_d = nc.declare_dram_parameter("wq", [HPC, H], f32, isOutput=False)
    wk_d = nc.declare_dram_parameter("wk", [HPC, H], f32, isOutput=False)
    wv_d = nc.declare_dram_parameter("wv", [HPC, H], f32, isOutput=False)
    bq_d = nc.declare_dram_parameter("bq", [HPC], f32, isOutput=False)
    bk_d = nc.declare_dram_parameter("bk", [HPC], f32, isOutput=False)
    bv_d = nc.declare_dram_parameter("bv", [HPC], f32, isOutput=False)
    mask_d = nc.declare_dram_parameter("mask", [S], f32, isOutput=False)
    out_d = nc.declare_dram_parameter("out", [S, HPC], f32, isOutput=True)

    with tile.TileContext(nc) as tc:
        import contextlib
        with contextlib.ExitStack() as stack:
            consts = stack.enter_context(tc.tile_pool(name="consts", bufs=1))
            p_qk = stack.enter_context(tc.tile_pool(name="p_qk", bufs=1))
            p_v = stack.enter_context(tc.tile_pool(name="p_v", bufs=1))
            p_e = stack.enter_context(tc.tile_pool(name="p_e", bufs=3))
            p_ep = stack.enter_context(tc.tile_pool(name="p_ep", bufs=1))
            p_post = stack.enter_context(tc.tile_pool(name="p_post", bufs=1))
            p_rec = stack.enter_context(tc.tile_pool(name="p_rec", bufs=4))

            # ---- constants ----
            ident = consts.tile([128, 128], bf16)
            make_identity(nc, ident)
            ones_row = consts.tile([1, 128], bf16)
            nc.gpsimd.memset(ones_row, 1.0)
            ones6 = consts.tile([128, NHEADS], f32)
            nc.gpsimd.memset(ones6, 1.0)
            bq_sb = consts.tile([128, NPAIR], f32)
            nc.gpsimd.dma_start(out=bq_sb, in_=bq_d[:].rearrange("(t p) -> p t", p=128))
            bk_sb = consts.tile([128, NPAIR], f32)
            nc.gpsimd.dma_start(out=bk_sb, in_=bk_d[:].rearrange("(t p) -> p t", p=128))
            bv_f32 = consts.tile([1, HPC], f32)
            nc.gpsimd.dma_start(out=bv_f32, in_=bv_d[:].rearrange("(o s) -> o s", o=1))
            bv_sb = consts.tile([1, HPC], bf16)
            nc.vector.tensor_copy(bv_sb, bv_f32)
            mask_sb = consts.tile([128, NST], f32)
            nc.gpsimd.dma_start(out=mask_sb, in_=mask_d[:].rearrange("(t p) -> p t", p=128))
            wmask = consts.tile([128, NST], f32)
            nc.scalar.activation(wmask, mask_sb, AF.Exp)

            # ---- persistent ----
            qt = p_qk.tile([128, NPAIR, S], bf16)      # Q^T pairs
            kt_sb = p_qk.tile([128, NPAIR, S], bf16)   # K^T pairs
            v_sb = p_v.tile([128, NST, NHEADS, HD + 1], bf16)
            out_sb = p_post.tile([128, NST, HPC], f32)

            def build():
                with tc.tile_pool(name="p_xw", bufs=1) as pxw, \
                        tc.tile_pool(name="p_nat", bufs=1) as pn, \
                        tc.tile_pool(name="psum_all", bufs=1, space="PSUM") as pa:
                    xt = pxw.tile([128, NHT, S], bf16)
                    wt_q = pxw.tile([128, NHT, HPC], bf16)
                    wt_k = pxw.tile([128, NHT, HPC], bf16)
                    wt_v = pxw.tile([128, NHT, HPC], bf16)

                    # ---- DMA everything up front (queues run in parallel).
                    # Priority order: wq/wk (prelude-critical), x st0..3,
                    # wv (needed ~first-exp time), then the x tail.
                    w_nat = {"q": [], "k": [], "v": []}

                    def dma_w(key, wd):
                        for ot in range(3):
                            wn = pn.tile([128, H], f32, tag="wnat",
                                         bufs=9, name="w_nat")
                            nc.gpsimd.dma_start(
                                out=wn, in_=wd[ot * 128:(ot + 1) * 128, :])
                            w_nat[key].append(wn)

                    dma_w("q", wq_d)
                    dma_w("k", wk_d)
                    x_nat = []
                    for st in range(NST):
                        t = pn.tile([128, H], f32, tag="xnat", bufs=12,
                                    name="x_nat")
                        nc.gpsimd.dma_start(out=t, in_=x_d[st * 128:(st + 1) * 128, :])
                        x_nat.append(t)
                        if st == 3:
                            dma_w("v", wv_d)

                    def s_tile(name):
                        return pa.tile([128, 2, 512], f32, tag="s", bufs=3,
                                       name=name)

                    xc_tiles = {}

                    def xt_item(stq, ht):
                        """Transpose x chunk stq, hidden tile ht -> xt."""
                        if ht == 0:
                            xc_tiles[stq] = []
                            for c in range(4):
                                xc = pn.tile([128, H], bf16, tag="xc", bufs=8,
                                             name="xc")
                                nc.vector.tensor_copy(xc, x_nat[stq * 4 + c])
                                xc_tiles[stq].append(xc)
                        ps = s_tile("tp_x").bitcast(bf16)
                        for c in range(4):
                            nc.tensor.matmul(
                                ps[:, 0, c * 128:(c + 1) * 128],
                                lhsT=xc_tiles[stq][c][:, ht * 128:(ht + 1) * 128],
                                rhs=ident, is_transpose=True,
                                start=(c == 0), stop=(c == 3))
                        nc.vector.tensor_copy(
                            xt[:, ht, stq * 512:(stq + 1) * 512], ps[:, 0, 0:512])

                    wc_tiles = {}

                    def wt_item(key, ht):
                        wt = {"q": wt_q, "k": wt_k, "v": wt_v}[key]
                        if ht == 0:
                            wc_tiles[key] = []
                            for ot in range(3):
                                wc = pn.tile([128, H], bf16, tag="wc", bufs=4,
                                             name="wc")
                                nc.vector.tensor_copy(wc, w_nat[key][ot])
                                wc_tiles[key].append(wc)
                        ps = s_tile("tp_w").bitcast(bf16)
                        for ot in range(3):
                            nc.tensor.matmul(
                                ps[:, 0, ot * 128:(ot + 1) * 128],
                                lhsT=wc_tiles[key][ot][:, ht * 128:(ht + 1) * 128],
                                rhs=ident, is_transpose=True,
                                start=(ot == 0), stop=(ot == 2))
                        nc.vector.tensor_copy(wt[:, ht, :], ps[:, 0, 0:HPC])

                    def qk_item(key, pair, sc):
                        """Project one [128,512] chunk of Q^T or K^T."""
                        wt, dst, bias = {
                            "q": (wt_q, qt, bq_sb),
                            "k": (wt_k, kt_sb, bk_sb)}[key]
                        ps = s_tile("pq")
                        pq = ps[:, 0, :]
                        for ht in range(NHT):
                            nc.tensor.matmul(
                                pq,
                                lhsT=wt[:, ht, pair * 128:(pair + 1) * 128],
                                rhs=xt[:, ht, sc * 512:(sc + 1) * 512],
                                start=(ht == 0), stop=(ht == NHT - 1))
                        nc.vector.tensor_scalar(
                            out=dst[:, pair, sc * 512:(sc + 1) * 512],
                            in0=pq, scalar1=bias[:, pair:pair + 1],
                            scalar2=None, op0=OP.add)

                    def v_item(st):
                        ps = s_tile("pv")
                        pv = ps[:, 0, 0:HPC]
                        for ht in range(NHT):
                            nc.tensor.matmul(
                                pv,
                                lhsT=xt[:, ht, st * 128:(st + 1) * 128],
                                rhs=wt_v[:, ht, :],
                                start=(ht == 0), stop=False)
                        nc.tensor.matmul(pv, lhsT=ones_row, rhs=bv_sb,
                                         start=False, stop=True)
                        nc.vector.tensor_scalar(
                            out=v_sb[:, st, :, 0:HD],
                            in0=pv.rearrange("p (h d) -> p h d", h=NHEADS),
                            scalar1=wmask[:, st:st + 1], scalar2=None, op0=OP.mult)
                        nc.vector.tensor_scalar(
                            out=v_sb[:, st, :, HD], in0=ones6,
                            scalar1=wmask[:, st:st + 1], scalar2=None, op0=OP.mult)

                    # ---- prelude: minimum work before attention can start --
                    for ht in range(NHT):
                        wt_item("k", ht)
                    for ht in range(NHT):
                        xt_item(0, ht)
                    qk_item("k", 0, 0)
                    for ht in range(NHT):
                        wt_item("q", ht)
                    qk_item("q", 0, 0)

                    # ---- deferred work, injected into attention slots ------
                    # inject[unit][ktp] -> list of closures; deadlines:
                    #   k-chunk sc by end of ktp 2sc-1, v st by ktp st//2,
                    #   q-chunk qtr(pair0) before unit qtr, pair p by unit 4p.
                    def I(fn, *a):
                        return lambda: fn(*a)

                    inject = {
                        (0, 0): [I(wt_item, "v", h) for h in range(NHT)] +
                                [I(v_item, 0), I(v_item, 1)] +
                                [I(xt_item, 1, h) for h in range(3)],
                        (0, 1): [I(xt_item, 1, h) for h in range(3, 6)] +
                                [I(qk_item, "k", 0, 1), I(v_item, 2), I(v_item, 3)],
                        (0, 2): [I(v_item, 4), I(v_item, 5)] +
                                [I(xt_item, 2, h) for h in range(4)],
                        (0, 3): [I(xt_item, 2, h) for h in range(4, 6)] +
                                [I(qk_item, "k", 0, 2), I(v_item, 6), I(v_item, 7)],
                        (0, 4): [I(v_item, 8), I(v_item, 9)] +
                                [I(xt_item, 3, h) for h in range(4)],
                        (0, 5): [I(xt_item, 3, h) for h in range(4, 6)] +
                                [I(qk_item, "k", 0, 3), I(v_item, 10), I(v_item, 11)],
                        (0, 6): [I(v_item, 12), I(v_item, 13), I(qk_item, "q", 0, 1)],
                        (0, 7): [I(v_item, 14), I(v_item, 15)],
                        (1, 0): [I(qk_item, "q", 0, 2)],
                        (1, 2): [I(qk_item, "q", 0, 3)],
                        (1, 4): [I(qk_item, "k", 1, 0)],
                        (1, 6): [I(qk_item, "k", 1, 1)],
                        (2, 0): [I(qk_item, "k", 1, 2)],
                        (2, 2): [I(qk_item, "k", 1, 3)],
                        (2, 4): [I(qk_item, "q", 1, 0)],
                        (2, 6): [I(qk_item, "q", 1, 1)],
                        (3, 0): [I(qk_item, "q", 1, 2)],
                        (3, 2): [I(qk_item, "q", 1, 3)],
                        (3, 4): [I(qk_item, "k", 2, 0)],
                        (3, 6): [I(qk_item, "k", 2, 1)],
                        (4, 0): [I(qk_item, "k", 2, 2)],
                        (4, 2): [I(qk_item, "k", 2, 3)],
                        (4, 4): [I(qk_item, "q", 2, 0)],
                        (4, 6): [I(qk_item, "q", 2, 1)],
                        (5, 0): [I(qk_item, "q", 2, 2)],
                        (5, 2): [I(qk_item, "q", 2, 3)],
                    }

                    # ---- attention: pair outer, qtr inner ------------------
                    def scores_mm(dst, pair, qs, ktp, hh):
                        """scoresT for head (2*pair+hh), k-tiles 2ktp,2ktp+1."""
                        lo, hi = 64 * hh, 64 * (hh + 1)
                        for par in range(2):
                            kt = 2 * ktp + par
                            nc.tensor.matmul(
                                dst[:, par, :],
                                lhsT=kt_sb[lo:hi, pair, kt * 128:(kt + 1) * 128],
                                rhs=qt[lo:hi, pair, qs:qs + 512],
                                start=True, stop=True)

                    def ctx_mm(ctx_t, e_t, pair, ktp, hh):
                        """ctxT[65, q] += v_aug^T @ E for head 2*pair+hh.

                        N=512 streams keep the PE weight loads (65 cols)
                        fully hidden; 4 matmuls/ktp instead of 16."""
                        head = 2 * pair + hh
                        for par in range(2):
                            kt = 2 * ktp + par
                            nc.tensor.matmul(
                                ctx_t,
                                lhsT=v_sb[:, kt, head, :],
                                rhs=e_t[:, par, :],
                                start=(kt == 0), stop=(kt == NST - 1))

                    for pair in range(NPAIR):
                        for qtr in range(4):
                            unit = pair * 4 + qtr
                            qs = qtr * 512
                            ctx_a = pa.tile([65, 512], f32, tag="ctx",
                                            bufs=2, name="ctx_a")
                            ctx_b = pa.tile([65, 512], f32, tag="ctx",
                                            bufs=2, name="ctx_b")
                            prev_eb = None
                            for ktp in range(8):
                                sa = s_tile("sa")
                                scores_mm(sa, pair, qs, ktp, 0)
                                if prev_eb is not None:
                                    ctx_mm(ctx_b, prev_eb, pair, ktp - 1, 1)
                                sb_ = s_tile("sb_")
                                scores_mm(sb_, pair, qs, ktp, 1)
                                e_a = p_e.tile([128, 2, 512], bf16, tag="e",
                                               bufs=4, name="e_a")
                                e_b = p_e.tile([128, 2, 512], bf16, tag="e",
                                               bufs=4, name="e_b")
                                nc.scalar.activation(e_a, sa, AF.Exp,
                                                     scale=float(SCALE))
                                nc.scalar.activation(e_b, sb_, AF.Exp,
                                                     scale=float(SCALE))
                                for item in inject.pop((unit, ktp), ()):
                                    item()
                                ctx_mm(ctx_a, e_a, pair, ktp, 0)
                                prev_eb = e_b
                            ctx_mm(ctx_b, prev_eb, pair, 7, 1)

                            # epilogue: copy ctxT to SBUF, transpose back to
                            # natural [q, d] via spare "s" PSUM, normalize.
                            for hh, ctx_t in ((0, ctx_a), (1, ctx_b)):
                                head = 2 * pair + hh
                                csb = p_ep.tile([65, 512], bf16, tag="csb",
                                                bufs=4, name="csb")
                                nc.vector.tensor_copy(csb, ctx_t)
                                ps = s_tile("ep").bitcast(bf16)
                                psr = ps[:, 0, 0:512].rearrange(
                                    "p (j c) -> p j c", c=128)
                                for j in range(4):
                                    nc.tensor.matmul(
                                        psr[:, j, 0:HD + 1],
                                        lhsT=csb[:, j * 128:(j + 1) * 128],
                                        rhs=ident[0:HD + 1, 0:HD + 1],
                                        is_transpose=True,
                                        start=(j == 0), stop=(j == 3))
                                rec = p_rec.tile([128, 4, 1], f32, tag="rec",
                                                 name="rec")
                                nc.vector.reciprocal(rec, psr[:, :, HD:HD + 1])
                                for j in range(4):
                                    st = qtr * 4 + j
                                    nc.vector.tensor_scalar(
                                        out=out_sb[:, st, head * HD:(head + 1) * HD],
                                        in0=psr[:, j, 0:HD],
                                        scalar1=rec[:, j, :],
                                        scalar2=None, op0=OP.mult)
                            if pair == NPAIR - 1:
                                for j in range(4):
                                    st = qtr * 4 + j
                                    nc.gpsimd.dma_start(
                                        out=out_d[st * 128:(st + 1) * 128, :],
                                        in_=out_sb[:, st, :])
                    assert not inject, f"unconsumed inject items: {list(inject)}"

            if reps == 1:
                build()
            else:
                with tc.For_i(0, reps, 1):
                    build()

    nc.compile()
    return nc


def make_runner(nc):
    """jit-compiled shard_map runner over 8 cores."""
    import jax
    import numpy as _np
    from jax.sharding import Mesh, NamedSharding, PartitionSpec
    from jax.experimental.shard_map import shard_map
    import concourse.mybir as mybir
    from concourse.bass2jax import (_bass_exec_p, install_neuronx_cc_hook,
                                    partition_id_tensor)

    install_neuronx_cc_hook()
    part_name = nc.partition_id_tensor.name if nc.partition_id_tensor else None
    in_names, out_names, out_avals, out_shapes = [], [], [], []
    for alloc in nc.m.functions[0].allocations:
        if not isinstance(alloc, mybir.MemoryLocationSet):
            continue
        name = alloc.memorylocations[0].name
        if alloc.kind == "ExternalInput":
            if name != part_name:
                in_names.append(name)
        elif alloc.kind == "ExternalOutput":
            out_names.append(name)
            shape = tuple(alloc.tensor_shape)
            dtype = mybir.dt.np(alloc.dtype)
            out_avals.append(jax.core.ShapedArray(shape, dtype))
            out_shapes.append((shape, dtype))
    n_params = len(in_names)
    all_in_names = list(in_names) + list(out_names)
    if part_name is not None:
        all_in_names.append(part_name)

    def _body(*args):
        operands = list(args)
        if part_name is not None:
            operands.append(partition_id_tensor())
        outs = _bass_exec_p.bind(
            *operands,
            out_avals=tuple(out_avals),
            in_names=tuple(all_in_names),
            out_names=tuple(out_names),
            lowering_input_output_aliases=(),
            sim_require_finite=True,
            sim_require_nnan=True,
            nc=nc,
        )
        return tuple(outs)

    devices = jax.devices()[:NCORES]
    mesh = Mesh(_np.asarray(devices), ("core",))
    sharded = jax.jit(
        shard_map(_body, mesh=mesh,
                  in_specs=(PartitionSpec("core"),) * (n_params + len(out_names)),
                  out_specs=(PartitionSpec("core"),) * len(out_names),
                  check_rep=False),
        keep_unused=True)
    sh = NamedSharding(mesh, PartitionSpec("core"))

    def stage(in_maps):
        import jax as _jax
        concat_in = [
            _jax.device_put(
                _np.ascontiguousarray(
                    _np.concatenate([_np.asarray(m[name]) for m in in_maps], axis=0)),
                sh)
            for name in in_names]
        concat_zeros = [
            _jax.device_put(_np.zeros((NCORES * sh_[0], *sh_[1:]), dt), sh)
            for (sh_, dt) in out_shapes]
        return concat_in, concat_zeros

    def run(concat_in, concat_zeros):
        import jax as _jax
        outs = sharded(*concat_in, *concat_zeros)
        _jax.block_until_ready(outs)
        return outs

    def unpack(outs):
        res = []
        for c in range(NCORES):
            m = {}
            for i, name in enumerate(out_names):
                shape, dt = out_shapes[i]
                m[name] = np.asarray(outs[i]).reshape(NCORES, *shape)[c]
            res.append(m)
        return res

    return stage, run, unpack


def shard_inputs(hidden_states, attention_mask, Wq, bq, Wk, bk, Wv, bv):
    hs = np.asarray(hidden_states, dtype=np.float32)
    am = np.asarray(attention_mask, dtype=np.float32)
    Wq, Wk, Wv = (np.asarray(w, dtype=np.float32) for w in (Wq, Wk, Wv))
    bq, bk, bv = (np.asarray(b, dtype=np.float32) for b in (bq, bk, bv))
    in_maps = []
    for c in range(NCORES):
        b = c // 2
        g = c % 2
        rows = slice(g * HPC, (g + 1) * HPC)
        in_maps.append({
            "x": np.ascontiguousarray(hs[b]),
            "wq": np.ascontiguousarray(Wq[rows]),
            "wk": np.ascontiguousarray(Wk[rows]),
            "wv": np.ascontiguousarray(Wv[rows]),
            "bq": np.ascontiguousarray(bq[rows]),
            "bk": np.ascontiguousarray(bk[rows]),
            "bv": np.ascontiguousarray(bv[rows]),
            "mask": np.ascontiguousarray(am[b, 0, 0, :]),
        })
    return in_maps


def unshard_outputs(results):
    out = np.empty((B, S, H), dtype=np.float32)
    for c in range(NCORES):
        b = c // 2
        g = c % 2
        out[b][:, g * HPC:(g + 1) * HPC] = results[c]["out"]
    return out


def get_compiled(reps=1, loop_part="all"):
    key = (reps, loop_part)
    if key in _CACHE:
        return _CACHE[key]
    if True:
        nc = build_nc(reps, loop_part)
        stage, run, unpack = make_runner(nc)
        _CACHE[key] = (nc, stage, run, unpack)
    return _CACHE[key]


def kernel(hidden_states, attention_mask, Wq, bq, Wk, bk, Wv, bv):
    _, stage, run, unpack = get_compiled(reps=1)
    in_maps = shard_inputs(hidden_states, attention_mask, Wq, bq, Wk, bk, Wv, bv)
    ci, cz = stage(in_maps)
    outs = run(ci, cz)
    return unshard_outputs(unpack(outs))


# revision 19
# speedup vs baseline: 1.7206x; 1.7206x over previous
"""BertSelfAttention TRN2 kernel (v2 — ACT-engine-saturated design).

Problem: B=4, S=2048, H=768, NH=12, HD=64, fp32.
Sharding: 8 cores; core c owns batch b = c//2 and head-group g = c%2
(6 heads = 384 hidden cols). Each core runs the same Bass program on its
shard; host reassembles.

Bottleneck analysis: the softmax exp is 6*S^2 = 25.2M elements per core
and can only run on the Activation engine (1 elem/cycle/partition
@1.2GHz) => ~199us floor. Everything else is organized to hide under
that: all matmuls in bf16 (1 col/cycle on PE), context accumulated in
natural [q, d] layout with a fused denominator column (no output
transposes), and the emission order software-pipelines scores(k+1)
between the two ctx halves of k so ACT never waits on PSUM buffers.

Per-core algorithm (fp32 PSUM, bf16 operands):
  xt = x^T, wt = W^T          (PE transposes of bf16 casts)
  QT/KT[pair] = W @ xT + b    (partitions = [headA d | headB d])
  V = x @ WvT + bv, scaled by w_k = exp(mask_k); augmented col w
  per (qtr, pair, ktp):
     scoresT[k, q] = K @ QT   (2 heads x 2 k-tiles, N=512)
     E = exp(SCALE * scoresT) (ACT, [128,1024] from PSUM -> bf16 SBUF)
     ctx[q, 65] += E^T @ v_aug  (natural layout, denom in col 64)
  epilogue: rec = 1/ctx[:,64]; out = ctx[:,0:64] * rec  (DVE only)
"""

import numpy as np

B, S, H = 4, 2048, 768
NH, HD = 12, 64
SCALE = 1.0 / np.sqrt(np.float32(HD)).astype(np.float32)
HPC = H // 2          # 384 hidden cols per core (6 heads)
NHEADS = 6            # heads per core
NPAIR = 3             # head pairs per core
NST = S // 128        # 16 S-tiles
NHT = H // 128        # 6 hidden tiles
NCORES = 8

_CACHE = {}


def build_nc(reps=1, loop_part="all"):
    import concourse.bacc as bacc
    import concourse.mybir as mybir
    import concourse.tile as tile
    from concourse.masks import make_identity

    f32 = mybir.dt.float32
    bf16 = mybir.dt.bfloat16
    f8 = mybir.dt.float8e4
    AF = mybir.ActivationFunctionType
    OP = mybir.AluOpType

    nc = bacc.Bacc("TRN2", target_bir_lowering=False, debug=False,
                   num_devices=NCORES)

    x_d = nc.declare_dram_parameter("x", [S, H], f32, isOutput=False)
    wq_d = nc.declare_dram_parameter("wq", [HPC, H], f32, isOutput=False)
    wk_d = nc.declare_dram_parameter("wk", [HPC, H], f32, isOutput=False)
    wv_d = nc.declare_dram_parameter("wv", [HPC, H], f32, isOutput=False)
    bq_d = nc.declare_dram_parameter("bq", [HPC], f32, isOutput=False)
    bk_d = nc.declare_dram_parameter("bk", [HPC], f32, isOutput=False)
    bv_d = nc.declare_dram_parameter("bv", [HPC], f32, isOutput=False)
    mask_d = nc.declare_dram_parameter("mask", [S], f32, isOutput=False)
    out_d = nc.declare_dram_parameter("out", [S, HPC], f32, isOutput=True)

    with tile.TileContext(nc) as tc:
        import contextlib
        with contextlib.ExitStack() as stack:
            consts = stack.enter_context(tc.tile_pool(name="consts", bufs=1))
            p_qk = stack.enter_context(tc.tile_pool(name="p_qk", bufs=1))
            p_v = stack.enter_context(tc.tile_pool(name="p_v", bufs=1))
            p_e = stack.enter_context(tc.tile_pool(name="p_e", bufs=3))
            p_post = stack.enter_context(tc.tile_pool(name="p_post", bufs=1))
            p_rec = stack.enter_context(tc.tile_pool(name="p_rec", bufs=4))

            # ---- constants ----
            ident = consts.tile([128, 128], bf16)
            make_identity(nc, ident)
            ones_row = consts.tile([1, 128], bf16)
            nc.gpsimd.memset(ones_row, 1.0)
            ones6 = consts.tile([128, NHEADS], f32)
            nc.gpsimd.memset(ones6, 1.0)
            bq_sb = consts.tile([128, NPAIR], f32)
            nc.gpsimd.dma_start(out=bq_sb, in_=bq_d[:].rearrange("(t p) -> p t", p=128))
            bk_sb = consts.tile([128, NPAIR], f32)
            nc.gpsimd.dma_start(out=bk_sb, in_=bk_d[:].rearrange("(t p) -> p t", p=128))
            bv_f32 = consts.tile([1, HPC], f32)
            nc.gpsimd.dma_start(out=bv_f32, in_=bv_d[:].rearrange("(o s) -> o s", o=1))
            bv_sb = consts.tile([1, HPC], bf16)
            nc.vector.tensor_copy(bv_sb, bv_f32)
            mask_sb = consts.tile([128, NST], f32)
            nc.gpsimd.dma_start(out=mask_sb, in_=mask_d[:].rearrange("(t p) -> p t", p=128))
            wmask = consts.tile([128, NST], f32)
            nc.scalar.activation(wmask, mask_sb, AF.Exp)

            # ---- persistent ----
            qt = p_qk.tile([128, NPAIR, S], bf16)      # Q^T pairs
            kt_sb = p_qk.tile([128, NPAIR, S], bf16)   # K^T pairs
            v_sb = p_v.tile([128, NST, NHEADS, HD + 1], bf16)
            out_sb = p_post.tile([128, NST, HPC], f32)

            def build():
                with tc.tile_pool(name="p_xw", bufs=1) as pxw, \
                        tc.tile_pool(name="p_nat", bufs=1) as pn, \
                        tc.tile_pool(name="psum_all", bufs=1, space="PSUM") as pa:
                    xt = pxw.tile([128, NHT, S], bf16)
                    wt_q = pxw.tile([128, NHT, HPC], bf16)
                    wt_k = pxw.tile([128, NHT, HPC], bf16)
                    wt_v = pxw.tile([128, NHT, HPC], bf16)

                    # ---- DMA everything up front, batched: one strided
                    # transfer per 4 x-tiles / per weight matrix (dma_start
                    # costs ~1.2us of Pool-engine time each, so fewer is
                    # faster). Priority: wq/wk, x group 0, wv, x tail.
                    w_nat = {}

                    def dma_w(key, wd):
                        wn = pn.tile([128, 3, H], f32, tag="wnat",
                                     bufs=3, name="w_nat")
                        nc.gpsimd.dma_start(
                            out=wn, in_=wd[:, :].rearrange("(c p) h -> p c h", p=128))
                        w_nat[key] = wn

                    dma_w("k", wk_d)
                    dma_w("q", wq_d)
                    x0_nat = []
                    for st in range(4):
                        t = pn.tile([128, H], f32, tag="x0nat", bufs=4,
                                    name="x0_nat")
                        nc.gpsimd.dma_start(
                            out=t, in_=x_d[st * 128:(st + 1) * 128, :])
                        x0_nat.append(t)
                    dma_w("v", wv_d)
                    x_nat = [None]
                    for g in range(1, 4):
                        t = pn.tile([128, 4, H], f32, tag="xnat", bufs=3,
                                    name="x_nat")
                        nc.gpsimd.dma_start(
                            out=t,
                            in_=x_d[g * 512:(g + 1) * 512, :].rearrange(
                                "(c p) h -> p c h", p=128))
                        x_nat.append(t)

                    def s_tile(name):
                        return pa.tile([128, 2, 512], f32, tag="s", bufs=3,
                                       name=name)

                    xc_tiles = {}

                    def xt_item(stq, ht):
                        """Transpose x chunk stq, hidden tile ht -> xt."""
                        if ht == 0:
                            xc = pn.tile([128, 4, H], bf16, tag="xc", bufs=2,
                                         name="xc")
                            if stq == 0:
                                for c in range(4):
                                    nc.vector.tensor_copy(xc[:, c, :], x0_nat[c])
                            else:
                                nc.vector.tensor_copy(xc, x_nat[stq])
                            xc_tiles[stq] = xc
                        ps = s_tile("tp_x").bitcast(bf16)
                        for c in range(4):
                            nc.tensor.matmul(
                                ps[:, 0, c * 128:(c + 1) * 128],
                                lhsT=xc_tiles[stq][:, c, ht * 128:(ht + 1) * 128],
                                rhs=ident, is_transpose=True,
                                start=(c == 0), stop=(c == 3))
                        nc.vector.tensor_copy(
                            xt[:, ht, stq * 512:(stq + 1) * 512], ps[:, 0, 0:512])

                    wc_tiles = {}

                    def wt_item(key, ht):
                        wt = {"q": wt_q, "k": wt_k, "v": wt_v}[key]
                        if ht == 0:
                            wc = pn.tile([128, 3, H], bf16, tag="wc", bufs=2,
                                         name="wc")
                            nc.vector.tensor_copy(wc, w_nat[key])
                            wc_tiles[key] = wc
                        ps = s_tile("tp_w").bitcast(bf16)
                        for ot in range(3):
                            nc.tensor.matmul(
                                ps[:, 0, ot * 128:(ot + 1) * 128],
                                lhsT=wc_tiles[key][:, ot, ht * 128:(ht + 1) * 128],
                                rhs=ident, is_transpose=True,
                                start=(ot == 0), stop=(ot == 2))
                        nc.vector.tensor_copy(wt[:, ht, :], ps[:, 0, 0:HPC])

                    def qk_item(key, pair, sc):
                        """Project one [128,512] chunk of Q^T or K^T."""
                        wt, dst, bias = {
                            "q": (wt_q, qt, bq_sb),
                            "k": (wt_k, kt_sb, bk_sb)}[key]
                        ps = s_tile("pq")
                        pq = ps[:, 0, :]
                        for ht in range(NHT):
                            nc.tensor.matmul(
                                pq,
                                lhsT=wt[:, ht, pair * 128:(pair + 1) * 128],
                                rhs=xt[:, ht, sc * 512:(sc + 1) * 512],
                                start=(ht == 0), stop=(ht == NHT - 1))
                        nc.vector.tensor_scalar(
                            out=dst[:, pair, sc * 512:(sc + 1) * 512],
                            in0=pq, scalar1=bias[:, pair:pair + 1],
                            scalar2=None, op0=OP.add)

                    def v_item(st):
                        ps = s_tile("pv")
                        pv = ps[:, 0, 0:HPC]
                        for ht in range(NHT):
                            nc.tensor.matmul(
                                pv,
                                lhsT=xt[:, ht, st * 128:(st + 1) * 128],
                                rhs=wt_v[:, ht, :],
                                start=(ht == 0), stop=False)
                        nc.tensor.matmul(pv, lhsT=ones_row, rhs=bv_sb,
                                         start=False, stop=True)
                        nc.vector.tensor_scalar(
                            out=v_sb[:, st, :, 0:HD],
                            in0=pv.rearrange("p (h d) -> p h d", h=NHEADS),
                            scalar1=wmask[:, st:st + 1], scalar2=None, op0=OP.mult)
                        nc.vector.tensor_scalar(
                            out=v_sb[:, st, :, HD], in0=ones6,
                            scalar1=wmask[:, st:st + 1], scalar2=None, op0=OP.mult)

                    # ---- prelude: minimum work before attention can start --
                    for ht in range(NHT):
                        wt_item("k", ht)
                    for ht in range(NHT):
                        xt_item(0, ht)
                    qk_item("k", 0, 0)
                    for ht in range(NHT):
                        wt_item("q", ht)
                    qk_item("q", 0, 0)

                    # ---- deferred work, injected into attention slots ------
                    # inject[unit][ktp] -> list of closures; deadlines:
                    #   k-chunk sc by end of ktp 2sc-1, v st by ktp st//2,
                    #   q-chunk qtr(pair0) before unit qtr, pair p by unit 4p.
                    def I(fn, *a):
                        return lambda: fn(*a)

                    inject = {
                        (0, 0): [I(wt_item, "v", h) for h in range(NHT)] +
                                [I(v_item, 0), I(v_item, 1)] +
                                [I(xt_item, 1, h) for h in range(3)],
                        (0, 1): [I(xt_item, 1, h) for h in range(3, 6)] +
                                [I(qk_item, "k", 0, 1), I(v_item, 2), I(v_item, 3)],
                        (0, 2): [I(v_item, 4), I(v_item, 5)] +
                                [I(xt_item, 2, h) for h in range(4)],
                        (0, 3): [I(xt_item, 2, h) for h in range(4, 6)] +
                                [I(qk_item, "k", 0, 2), I(v_item, 6), I(v_item, 7)],
                        (0, 4): [I(v_item, 8), I(v_item, 9)] +
                                [I(xt_item, 3, h) for h in range(4)],
                        (0, 5): [I(xt_item, 3, h) for h in range(4, 6)] +
                                [I(qk_item, "k", 0, 3), I(v_item, 10), I(v_item, 11)],
                        (0, 6): [I(v_item, 12), I(v_item, 13), I(qk_item, "q", 0, 1)],
                        (0, 7): [I(v_item, 14), I(v_item, 15)],
                        (1, 0): [I(qk_item, "q", 0, 2)],
                        (1, 2): [I(qk_item, "q", 0, 3)],
                        (1, 4): [I(qk_item, "k", 1, 0)],
                        (1, 6): [I(qk_item, "k", 1, 1)],
                        (2, 0): [I(qk_item, "k", 1, 2)],
                        (2, 2): [I(qk_item, "k", 1, 3)],
                        (2, 4): [I(qk_item, "q", 1, 0)],
                        (2, 6): [I(qk_item, "q", 1, 1)],
                        (3, 0): [I(qk_item, "q", 1, 2)],
                        (3, 2): [I(qk_item, "q", 1, 3)],
                        (3, 4): [I(qk_item, "k", 2, 0)],
                        (3, 6): [I(qk_item, "k", 2, 1)],
                        (4, 0): [I(qk_item, "k", 2, 2)],
                        (4, 2): [I(qk_item, "k", 2, 3)],
                        (4, 4): [I(qk_item, "q", 2, 0)],
                        (4, 6): [I(qk_item, "q", 2, 1)],
                        (5, 0): [I(qk_item, "q", 2, 2)],
                        (5, 2): [I(qk_item, "q", 2, 3)],
                    }

                    # ---- attention: pair outer, qtr inner ------------------
                    def scores_mm(dst, pair, qs, ktp, hh):
                        """scoresT for head (2*pair+hh), k-tiles 2ktp,2ktp+1."""
                        lo, hi = 64 * hh, 64 * (hh + 1)
                        for par in range(2):
                            kt = 2 * ktp + par
                            nc.tensor.matmul(
                                dst[:, par, :],
                                lhsT=kt_sb[lo:hi, pair, kt * 128:(kt + 1) * 128],
                                rhs=qt[lo:hi, pair, qs:qs + 512],
                                start=True, stop=True)

                    def ctx_mm(ctx_t, e_t, pair, ktp, hh):
                        """ctx[q,65] += E^T @ v_aug for head 2*pair+hh.

                        One PSUM bank holds all 4 j-regions; start/stop must
                        bracket the whole bank (lazy zero-region semantics),
                        so only the very first/last matmul set them."""
                        head = 2 * pair + hh
                        for par in range(2):
                            kt = 2 * ktp + par
                            for j in range(4):
                                nc.tensor.matmul(
                                    ctx_t[:, j, 0:HD + 1],
                                    lhsT=e_t[:, par, j * 128:(j + 1) * 128],
                                    rhs=v_sb[:, kt, head, :],
                                    start=(kt == 0 and j == 0),
                                    stop=(kt == NST - 1 and j == 3))

                    for pair in range(NPAIR):
                        for qtr in range(4):
                            unit = pair * 4 + qtr
                            qs = qtr * 512
                            ctx_a = pa.tile([128, 4, 128], f32, tag="ctx",
                                            bufs=2, name="ctx_a")
                            ctx_b = pa.tile([128, 4, 128], f32, tag="ctx",
                                            bufs=2, name="ctx_b")
                            prev_eb = None
                            for ktp in range(8):
                                sa = s_tile("sa")
                                scores_mm(sa, pair, qs, ktp, 0)
                                if prev_eb is not None:
                                    ctx_mm(ctx_b, prev_eb, pair, ktp - 1, 1)
                                sb_ = s_tile("sb_")
                                scores_mm(sb_, pair, qs, ktp, 1)
                                e_a = p_e.tile([128, 2, 512], bf16, tag="e",
                                               bufs=4, name="e_a")
                                e_b = p_e.tile([128, 2, 512], bf16, tag="e",
                                               bufs=4, name="e_b")
                                nc.scalar.activation(e_a, sa, AF.Exp,
                                                     scale=float(SCALE))
                                nc.scalar.activation(e_b, sb_, AF.Exp,
                                                     scale=float(SCALE))
                                for item in inject.pop((unit, ktp), ()):
                                    item()
                                ctx_mm(ctx_a, e_a, pair, ktp, 0)
                                prev_eb = e_b
                            ctx_mm(ctx_b, prev_eb, pair, 7, 1)

                            # epilogue: normalize in natural layout (DVE only)
                            for hh, ctx_t in ((0, ctx_a), (1, ctx_b)):
                                head = 2 * pair + hh
                                rec = p_rec.tile([128, 4, 1], f32, tag="rec",
                                                 name="rec")
                                nc.vector.reciprocal(rec, ctx_t[:, :, HD:HD + 1])
                                for j in range(4):
                                    st = qtr * 4 + j
                                    nc.vector.tensor_scalar(
                                        out=out_sb[:, st, head * HD:(head + 1) * HD],
                                        in0=ctx_t[:, j, 0:HD],
                                        scalar1=rec[:, j, :],
                                        scalar2=None, op0=OP.mult)
                            if pair == NPAIR - 1:
                                nc.gpsimd.dma_start(
                                    out=out_d[qtr * 512:(qtr + 1) * 512, :]
                                    .rearrange("(c p) h -> p c h", p=128),
                                    in_=out_sb[:, qtr * 4:(qtr + 1) * 4, :])
                    assert not inject, f"unconsumed inject items: {list(inject)}"

            if reps == 1:
                build()
            else:
                with tc.For_i(0, reps, 1):
                    build()

    nc.compile()
    return nc


def make_runner(nc):
    """jit-compiled shard_map runner over 8 cores."""
    import jax
    import numpy as _np
    from jax.sharding import Mesh, NamedSharding, PartitionSpec
    from jax.experimental.shard_map import shard_map
    import concourse.mybir as mybir
    from concourse.bass2jax import (_bass_exec_p, install_neuronx_cc_hook,
                                    partition_id_tensor)

    install_neuronx_cc_hook()
    part_name = nc.partition_id_tensor.name if nc.partition_id_tensor else None
    in_names, out_names, out_avals, out_shapes = [], [], [], []
    for alloc in nc.m.functions[0].allocations:
        if not isinstance(alloc, mybir.MemoryLocationSet):
            continue
        name = alloc.memorylocations[0].name
        if alloc.kind == "ExternalInput":
            if name != part_name:
                in_names.append(name)
        elif alloc.kind == "ExternalOutput":
            out_names.append(name)
            shape = tuple(alloc.tensor_shape)
            dtype = mybir.dt.np(alloc.dtype)
            out_avals.append(jax.core.ShapedArray(shape, dtype))
            out_shapes.append((shape, dtype))
    n_params = len(in_names)
    all_in_names = list(in_names) + list(out_names)
    if part_name is not None:
        all_in_names.append(part_name)

    def _body(*args):
        operands = list(args)
        if part_name is not None:
            operands.append(partition_id_tensor())
        outs = _bass_exec_p.bind(
            *operands,
            out_avals=tuple(out_avals),
            in_names=tuple(all_in_names),
            out_names=tuple(out_names),
            lowering_input_output_aliases=(),
            sim_require_finite=True,
            sim_require_nnan=True,
            nc=nc,
        )
        return tuple(outs)

    devices = jax.devices()[:NCORES]
    mesh = Mesh(_np.asarray(devices), ("core",))
    sharded = jax.jit(
        shard_map(_body, mesh=mesh,
                  in_specs=(PartitionSpec("core"),) * (n_params + len(out_names)),
                  out_specs=(PartitionSpec("core"),) * len(out_names),
                  check_rep=False),
        keep_unused=True)
    sh = NamedSharding(mesh, PartitionSpec("core"))

    def stage(in_maps):
        import jax as _jax
        concat_in = [
            _jax.device_put(
                _np.ascontiguousarray(
                    _np.concatenate([_np.asarray(m[name]) for m in in_maps], axis=0)),
                sh)
            for name in in_names]
        concat_zeros = [
            _jax.device_put(_np.zeros((NCORES * sh_[0], *sh_[1:]), dt), sh)
            for (sh_, dt) in out_shapes]
        return concat_in, concat_zeros

    def run(concat_in, concat_zeros):
        import jax as _jax
        outs = sharded(*concat_in, *concat_zeros)
        _jax.block_until_ready(outs)
        return outs

    def unpack(outs):
        res = []
        for c in range(NCORES):
            m = {}
            for i, name in enumerate(out_names):
                shape, dt = out_shapes[i]
                m[name] = np.asarray(outs[i]).reshape(NCORES, *shape)[c]
            res.append(m)
        return res

    return stage, run, unpack


def shard_inputs(hidden_states, attention_mask, Wq, bq, Wk, bk, Wv, bv):
    hs = np.asarray(hidden_states, dtype=np.float32)
    am = np.asarray(attention_mask, dtype=np.float32)
    Wq, Wk, Wv = (np.asarray(w, dtype=np.float32) for w in (Wq, Wk, Wv))
    bq, bk, bv = (np.asarray(b, dtype=np.float32) for b in (bq, bk, bv))
    in_maps = []
    for c in range(NCORES):
        b = c // 2
        g = c % 2
        rows = slice(g * HPC, (g + 1) * HPC)
        in_maps.append({
            "x": np.ascontiguousarray(hs[b]),
            "wq": np.ascontiguousarray(Wq[rows]),
            "wk": np.ascontiguousarray(Wk[rows]),
            "wv": np.ascontiguousarray(Wv[rows]),
            "bq": np.ascontiguousarray(bq[rows]),
            "bk": np.ascontiguousarray(bk[rows]),
            "bv": np.ascontiguousarray(bv[rows]),
            "mask": np.ascontiguousarray(am[b, 0, 0, :]),
        })
    return in_maps


def unshard_outputs(results):
    out = np.empty((B, S, H), dtype=np.float32)
    for c in range(NCORES):
        b = c // 2
        g = c % 2
        out[b][:, g * HPC:(g + 1) * HPC] = results[c]["out"]
    return out


def get_compiled(reps=1, loop_part="all"):
    key = (reps, loop_part)
    if key in _CACHE:
        return _CACHE[key]
    if True:
        nc = build_nc(reps, loop_part)
        stage, run, unpack = make_runner(nc)
        _CACHE[key] = (nc, stage, run, unpack)
    return _CACHE[key]


def kernel(hidden_states, attention_mask, Wq, bq, Wk, bk, Wv, bv):
    _, stage, run, unpack = get_compiled(reps=1)
    in_maps = shard_inputs(hidden_states, attention_mask, Wq, bq, Wk, bk, Wv, bv)
    ci, cz = stage(in_maps)
    outs = run(ci, cz)
    return unshard_outputs(unpack(outs))
